# revision 3
# baseline (speedup 1.0000x reference)
"""Dirichlet MLE (EstDirichlet) Trainium2 kernel, v2.

Full-input contract: kernel(x) takes the complete x [2_000_000, 10] f32 and
returns the fitted Dirichlet alpha [10] f32.

The Newton fixed point  digamma(a_c) - digamma(sum a) = logp_c  depends only
on logp = colmean(x) - mean_i log s_i with s_i = sum_c exp(x_ic).  The device
computes L = sum_i log s_i (data-parallel rows, 8 cores); the host does the
tiny 10-dim Newton solve plus subsampled moment/bias estimation.

v2 design (from v1's 32us; v1 notes preserved in kernel_v1_backup.py):
- ALL input rides fp8_e4m3 (1 B/elem, 2.52 MB/core): exp is split between
  ScalarE's exact exp (1 elem/cyc/lane, dtype-independent) and VectorE's
  Schraudolph int-exp, ONE tensor_scalar fp8 -> int16 whose rne_i16(x*EA+EB)
  IS the bf16 bit pattern of e^x (2x_2P mode for fp8 input).
- channel-major group tiles E[128, 10, G]: 10-channel row-sum is a 4-op
  FLAT tree on contiguous slices (u=E0:5+E5:10; v=u0:2+u2:4; w=v0+v1;
  s=w+u4), all DVE 2x.
- ON-DEVICE log: s bf16 bitcast to i16 is 128*(127+log2 s+sawtooth); one
  tensor_scalar affine (i16 -> bf16, 4x) with accum_out gives per-partition
  SUMS of log s for free.  Output per group: [128, 1] f32 -> the whole
  output is ~3 KB/core instead of 1 MB.
- calibration WITHOUT per-row outputs: groups 0/1 are small pure-path
  groups (ACT-only / DVE-only).  delta_path = (device aggregate - exact
  host logsumexp over those rows)/n_cal captures the ENTIRE pipeline bias
  (fp8 quant, exp path, bf16 tree, int-log sawtooth), applied per path to
  the full L.  Padded rows (zero-filled tail columns of core 7, col-major
  row map keeps them whole-column) contribute an exactly host-replicable
  constant per path.
- cfg knobs: op3/op4 of the tree on vector or gpsimd engine; d_cast routes
  DVE-path input through a SWDGE (gpsimd) casting DMA fp8->bf16 so the
  int-exp tensor_scalar runs at 4x instead of 2x.

HW-trace facts that shaped this: input DMA sustains ~318 GB/s while busy
(4.47 MB -> 14.4 us in v1; v2 moves 2.52 MB); ACT costs ~(FD+352)/1.2 ns
per ACTIVATE, dtype-independent; DVE tensor_scalar 2-byte in/out step-1
hits 4x, fp8-in 2x, tensor_tensor 2x; a fixed ~6 us preamble (engine iram
loads) + ~9 us BSP teardown sweep floors every measurement.
"""


import numpy as np
import ml_dtypes
from contextlib import ExitStack

import concourse.bass as bass
import concourse.tile as tile
from concourse import bacc, mybir
from concourse.bass_utils import run_bass_kernel_spmd

BF16 = mybir.dt.bfloat16
F32 = mybir.dt.float32
I16 = mybir.dt.int16
FP8 = mybir.dt.float8e4
NP_BF16 = ml_dtypes.bfloat16
NP_FP8 = ml_dtypes.float8_e4m3fn

N_CORES = 8
C = 10
N_ROWS = 2_000_000

N_ITERS = 200
TOL = 1e-10
SUBSAMPLE = 10

EA = float(np.float32(128.0 / np.log(2.0)))
EB = float(np.float32(128.0 * 127.0 - 7.3365))
C1 = float(np.float32(np.log(2.0) / 128.0))
C2 = float(np.float32(-(127.0 + 0.0430) * np.log(2.0)))

ADD = mybir.AluOpType.add
MULT = mybir.AluOpType.mult


def make_geom(groups, dma_order=None, dve_order=None, act_order=None):
    """groups: list of dicts(G, wa, a_pieces, d_pieces); groups 0/1 must be
    pure-A / pure-D (calibration).  dma_order: ('a'|'d', g, j) in sync-queue
    order.  dve_order: ('d', g, j) exps and ('t', g) trees in DVE program
    order.  act_order: (g, j) ACT piece order."""
    gs = []
    for gr in groups:
        G, wa = gr["G"], gr["wa"]
        a_pieces = gr.get("a_pieces") or ([wa] if wa else [])
        d_pieces = gr.get("d_pieces") or ([G - wa] if G - wa else [])
        assert sum(a_pieces) == wa and sum(d_pieces) == G - wa
        gs.append(dict(G=G, wa=wa, a_pieces=a_pieces, d_pieces=d_pieces))
    assert gs[0]["wa"] == gs[0]["G"] and gs[1]["wa"] == 0
    k = sum(g["G"] for g in gs)
    if dma_order is None:
        dma_order = []
        for g, gr in enumerate(gs):
            for j in range(len(gr["a_pieces"])):
                dma_order.append(("a", g, j))
            for j in range(len(gr["d_pieces"])):
                dma_order.append(("d", g, j))
    if dve_order is None:
        dve_order = [("d", g, j) for g, gr in enumerate(gs)
                     for j in range(len(gr["d_pieces"]))]
        dve_order += [("t", g) for g in range(len(gs))]
    if act_order is None:
        act_order = [(g, j) for g, gr in enumerate(gs)
                     for j in range(len(gr["a_pieces"]))]
    return dict(groups=gs, k=k, rows=128 * k, dma_order=list(dma_order),
                dve_order=list(dve_order), act_order=list(act_order))


GEOM_FULL = make_geom(
    [
        dict(G=64, wa=64),                                     # g0 cal-A
        dict(G=64, wa=0),                                      # g1 cal-D
        dict(G=512, wa=384, a_pieces=[192, 192], d_pieces=[128]),
        dict(G=576, wa=448, a_pieces=[224, 224], d_pieces=[128]),
        dict(G=512, wa=384, a_pieces=[384], d_pieces=[128]),
        dict(G=240, wa=160, a_pieces=[160], d_pieces=[80]),
    ],
    dma_order=[
        ("a", 0, 0), ("d", 1, 0), ("a", 2, 0), ("d", 2, 0),
        ("a", 2, 1), ("d", 3, 0), ("a", 3, 0), ("d", 4, 0),
        ("a", 3, 1), ("d", 5, 0), ("a", 4, 0), ("a", 5, 0),
    ],
    dve_order=[
        ("d", 1, 0), ("t", 1), ("d", 2, 0), ("d", 3, 0), ("d", 4, 0),
        ("d", 5, 0), ("t", 0), ("t", 2), ("t", 3), ("t", 4), ("t", 5),
    ],
)

CFG_DEFAULT = dict(op3="v", op4="v", d_cast=False)

_CACHE = {}


def emit_program(tc, ctx, aps, geom, cfg):
    nc = tc.nc
    gs = geom["groups"]
    x_d, acc_d = aps["xa"], aps["acc"]
    ng = len(gs)

    xa_pool = ctx.enter_context(tc.tile_pool(name="xa", bufs=1))
    e_pool = ctx.enter_context(tc.tile_pool(name="e", bufs=1))
    u_pool = ctx.enter_context(tc.tile_pool(name="u", bufs=1))
    v_pool = ctx.enter_context(tc.tile_pool(name="v", bufs=1))
    s_pool = ctx.enter_context(tc.tile_pool(name="s", bufs=1))
    acc_pool = ctx.enter_context(tc.tile_pool(name="acc", bufs=1))

    # dram offsets per (type, g, j), bytes in units of fp8 elems
    offs = {}
    o = 0
    for typ, g, j in geom["dma_order"]:
        w = gs[g]["a_pieces" if typ == "a" else "d_pieces"][j]
        offs[(typ, g, j)] = o
        o += 128 * C * w

    # SBUF tiles
    E, xt = {}, {}
    for g, gr in enumerate(gs):
        E[g] = e_pool.tile([128, C * gr["G"]], BF16, name=f"e{g}", tag=f"e{g}")
        for j, w in enumerate(gr["a_pieces"]):
            xt[("a", g, j)] = xa_pool.tile(
                [128, C * w], FP8, name=f"xa{g}_{j}", tag=f"xa{g}_{j}")
        for j, w in enumerate(gr["d_pieces"]):
            dt = BF16 if cfg["d_cast"] else FP8
            xt[("d", g, j)] = xa_pool.tile(
                [128, C * w], dt, name=f"xd{g}_{j}", tag=f"xd{g}_{j}")
    ACC = acc_pool.tile([128, ng], F32, name="acc", tag="acc")

    # 1) input DMAs in queue order; d-pieces go SWDGE (cast) when d_cast
    for typ, g, j in geom["dma_order"]:
        w = gs[g]["a_pieces" if typ == "a" else "d_pieces"][j]
        o = offs[(typ, g, j)]
        src = x_d[o : o + 128 * C * w].rearrange("(p f) -> p f", f=C * w)
        if typ == "d" and cfg["d_cast"]:
            nc.gpsimd.dma_start(xt[(typ, g, j)][:], src)
        else:
            nc.sync.dma_start(xt[(typ, g, j)][:], src)

    # 2) ACT exps in act_order
    for g, j in geom["act_order"]:
        gr = gs[g]
        G = gr["G"]
        w = gr["a_pieces"][j]
        o = sum(gr["a_pieces"][:j])
        E3 = E[g][:].rearrange("p (c t) -> p c t", t=G)
        nc.scalar.activation(
            E3[:, :, o : o + w],
            xt[("a", g, j)][:].rearrange("p (c t) -> p c t", t=w),
            mybir.ActivationFunctionType.Exp,
        )

    # 3) DVE program: int-exps and trees in dve_order
    for item in geom["dve_order"]:
        if item[0] == "d":
            _, g, j = item
            gr = gs[g]
            G = gr["G"]
            E3i = E[g][:].bitcast(I16).rearrange("p (c t) -> p c t", t=G)
            o = gr["wa"] + sum(gr["d_pieces"][:j])
            w = gr["d_pieces"][j]
            nc.vector.tensor_scalar(
                E3i[:, :, o : o + w],
                xt[("d", g, j)][:].rearrange("p (c t) -> p c t", t=w),
                EA, EB, op0=MULT, op1=ADD,
            )
        else:
            g = item[1]
            G = gs[g]["G"]
            U = u_pool.tile([128, 5 * G], BF16, name=f"u{g}", tag=f"u{g}")
            nc.vector.tensor_tensor(
                U[:], E[g][:, 0 : 5 * G], E[g][:, 5 * G : 10 * G], op=ADD)
            V = v_pool.tile([128, 2 * G], BF16, name=f"v{g}", tag=f"v{g}")
            nc.vector.tensor_tensor(
                V[:], U[:, 0 : 2 * G], U[:, 2 * G : 4 * G], op=ADD)
            S = s_pool.tile([128, 2 * G], BF16, name=f"s{g}", tag=f"s{g}")
            eng3 = nc.vector if cfg["op3"] == "v" else nc.gpsimd
            eng4 = nc.vector if cfg["op4"] == "v" else nc.gpsimd
            # w = v0 + v1 into S[0:G]; s = w + u4 into S[G:2G]
            eng3.tensor_tensor(S[:, 0:G], V[:, 0:G], V[:, G : 2 * G], op=ADD)
            eng4.tensor_tensor(
                S[:, G : 2 * G], S[:, 0:G], U[:, 4 * G : 5 * G], op=ADD)
            # int-log + per-partition accumulate; LS is a dummy main output.
            # accum_out semantics (HW-verified): out = bf16(in*s1) [op0 only],
            # accum = s2 + reduce_add(in*s1) in f32.  s2 = G*C2 makes the
            # accumulator exactly sum_t (i*C1 + C2) = sum_t intlog(s_t).
            LS = u_pool.tile([128, G], BF16, name=f"ls{g}", tag=f"ls{g}")
            nc.vector.tensor_scalar(
                LS[:], S[:, G : 2 * G].bitcast(I16), C1,
                float(np.float32(G * np.float32(C2))),
                op0=MULT, op1=ADD, accum_out=ACC[:, g : g + 1],
            )

    # 4) single tiny output DMA
    dst = acc_d[0 : 128 * ng].rearrange("(p f) -> p f", f=ng)
    nc.sync.dma_start(dst, ACC[:])


def build_nc(geom=None, cfg=None):
    geom = geom or GEOM_FULL
    cfg = cfg or CFG_DEFAULT
    key = str(geom) + str(cfg)
    if key in _CACHE:
        return _CACHE[key]
    nc = bacc.Bacc(
        "TRN2", target_bir_lowering=False, debug=False, num_devices=N_CORES
    )
    ntot = 128 * C * geom["k"]
    ng = len(geom["groups"])
    aps = {
        "xa": nc.dram_tensor("xa", [ntot], FP8, kind="ExternalInput").ap(),
        "acc": nc.dram_tensor(
            "acc", [128 * ng], F32, kind="ExternalOutput").ap(),
    }
    with tile.TileContext(nc) as tc, ExitStack() as ctx:
        emit_program(tc, ctx, aps, geom, cfg)
    nc.compile()
    _CACHE[key] = nc
    return nc


def shard_starts(n_rows, geom):
    r = geom["rows"]
    return [min(i * r, n_rows) for i in range(N_CORES)]


def pack_core(x, start, geom):
    gs = geom["groups"]
    k, r = geom["k"], geom["rows"]
    n_real = min(r, max(0, x.shape[0] - start))
    xr = np.zeros((r, C), dtype=np.float32)
    xr[:n_real] = x[start : start + n_real]
    # col-major row map: row = c*128 + p -> x3[p, ch, c]
    x3 = np.ascontiguousarray(xr.reshape(k, 128, C).transpose(1, 2, 0))
    goff = np.cumsum([0] + [g["G"] for g in gs])
    chunks = []
    for typ, g, j in geom["dma_order"]:
        gr = gs[g]
        if typ == "a":
            o = goff[g] + sum(gr["a_pieces"][:j])
            w = gr["a_pieces"][j]
        else:
            o = goff[g] + gr["wa"] + sum(gr["d_pieces"][:j])
            w = gr["d_pieces"][j]
        chunks.append(
            np.ascontiguousarray(x3[:, :, o : o + w]).reshape(-1))
    xa = np.concatenate(chunks).astype(NP_FP8)
    return xa, n_real


def digamma(x):
    x = np.asarray(x, dtype=np.float64)
    res = np.zeros_like(x)
    for i in range(8):
        res -= 1.0 / (x + i)
    y = x + 8.0
    y2 = 1.0 / (y * y)
    res += (
        np.log(y)
        - 0.5 / y
        - y2
        * (
            1.0 / 12
            - y2 * (1.0 / 120 - y2 * (1.0 / 252 - y2 * (1.0 / 240 - y2 / 132)))
        )
    )
    return res


def trigamma(x):
    x = np.asarray(x, dtype=np.float64)
    res = np.zeros_like(x)
    for i in range(8):
        res += 1.0 / (x + i) ** 2
    y = x + 8.0
    y2 = 1.0 / (y * y)
    res += (
        1.0 / y
        + 0.5 * y2
        + y2
        / y
        * (1.0 / 6 - y2 * (1.0 / 30 - y2 * (1.0 / 42 - y2 * (1.0 / 30 - y2 * 5.0 / 66))))
    )
    return res


def newton(m1, m2, logp, n):
    a = m1 * (((m1 - m2) / (m2 - m1 * m1)).mean())
    a = np.maximum(a, 1e-6)
    for _ in range(N_ITERS):
        asum = a.sum()
        g = (digamma(asum) - digamma(a) + logp) * n
        q = -n * trigamma(a)
        z = n * trigamma(asum)
        qi = 1.0 / q
        b = (g * qi).sum() / (1.0 / z + qi.sum())
        a_new = a - (g - b) * qi
        a_new = np.maximum(a_new, 1e-8)
        diff = np.abs(a_new - a).sum()
        a = a_new
        if diff < TOL:
            break
    return a


def run_device(x, geom=None, cfg=None, trace=False, **kw):
    geom = geom or GEOM_FULL
    cfg = cfg or CFG_DEFAULT
    nc = build_nc(geom, cfg)
    starts = shard_starts(x.shape[0], geom)
    in_maps = []
    for i in range(N_CORES):
        xa, _ = pack_core(x, starts[i], geom)
        in_maps.append({"xa": xa})
    res = run_bass_kernel_spmd(
        nc, in_maps, core_ids=list(range(N_CORES)), trace=trace, **kw
    )
    return res


def _bf16(x):
    return np.asarray(x, np.float32).astype(NP_BF16).astype(np.float32)


def pad_constants():
    """intlog of the padded-row (x=0) s per path, replicating device math."""
    eA = np.float32(1.0)
    iD = np.int16(np.rint(np.float32(0.0) * np.float32(EA) + np.float32(EB)))
    eD = iD.view(NP_BF16).astype(np.float32)
    out = {}
    for name, e in [("A", eA), ("D", eD)]:
        u = _bf16(e + e)
        v = _bf16(u + u)
        w = _bf16(v + v)
        s = _bf16(w + u)
        i = np.float32(np.asarray(s, np.float32).astype(NP_BF16).view(np.int16))
        out[name] = float(np.float32(i * np.float32(C1)) + np.float32(C2))
    return out


def col_path_map(geom):
    paths = []
    for gr in geom["groups"]:
        paths += [True] * gr["wa"] + [False] * (gr["G"] - gr["wa"])
    return np.array(paths)


def finish_host(x, results, geom=None):
    geom = geom or GEOM_FULL
    gs = geom["groups"]
    k = geom["k"]
    ng = len(gs)
    n = x.shape[0]
    starts = shard_starts(n, geom)
    paths = col_path_map(geom)
    pc = pad_constants()
    goff = np.cumsum([0] + [g["G"] for g in gs])

    L = 0.0
    n_A = 0
    n_D = 0
    cal_sums = {0: 0.0, 1: 0.0}
    cal_rows = {0: 0, 1: 0}
    for i in range(N_CORES):
        acc = np.asarray(results[i]["acc"]).reshape(128, ng)
        n_real = min(geom["rows"], max(0, n - starts[i]))
        rc = n_real // 128
        assert rc * 128 == n_real, (i, n_real)
        for g in range(ng):
            gsum = float(acc[:, g].sum(dtype=np.float64))
            c0, c1_ = goff[g], goff[g + 1]
            cols_real = np.arange(c0, c1_) < rc
            pa = int((~cols_real & paths[c0:c1_]).sum())
            pd = int((~cols_real & ~paths[c0:c1_]).sum())
            gsum -= 128 * (pa * pc["A"] + pd * pc["D"])
            L += gsum
            ra = int((cols_real & paths[c0:c1_]).sum())
            rd = int((cols_real & ~paths[c0:c1_]).sum())
            n_A += 128 * ra
            n_D += 128 * rd
            if g in (0, 1):
                cal_sums[g] += gsum
                cal_rows[g] += 128 * (ra + rd)
    assert n_A + n_D == n, (n_A, n_D)

    deltas = {}
    for g in (0, 1):
        c0 = goff[g]
        rows = []
        for i in range(N_CORES):
            st = starts[i] + c0 * 128
            rows.append(x[st : st + 128 * gs[g]["G"]])
        xr = np.concatenate(rows).astype(np.float64)
        m = xr.max(axis=1, keepdims=True)
        ls = np.log(np.exp(xr - m).sum(axis=1)) + m[:, 0]
        deltas[g] = cal_sums[g] / cal_rows[g] - ls.mean()
    L_corr = L - n_A * deltas[0] - n_D * deltas[1]

    xsum = x.sum(axis=0, dtype=np.float64)
    logp = xsum / n - L_corr / n

    xm = x[::SUBSAMPLE].astype(np.float64)
    es = np.exp(xm - xm.max(axis=1, keepdims=True))
    ps = es / es.sum(axis=1, keepdims=True)
    m1 = ps.mean(0)
    m2 = (ps * ps).mean(0)
    a = newton(m1, m2, logp, float(n))
    return a.astype(np.float32)


def kernel(x):
    x = np.asarray(x)
    assert x.shape == (N_ROWS, C) and x.dtype == np.float32, (x.shape, x.dtype)
    res = run_device(x)
    return finish_host(x, res.results)


# revision 6
# speedup vs baseline: 1.0053x; 1.0053x over previous
"""Dirichlet MLE (EstDirichlet) Trainium2 kernel, v3.

Full-input contract: kernel(x) takes the complete x [2_000_000, 10] f32 and
returns the fitted Dirichlet alpha [10] f32.

The Newton fixed point  digamma(a_c) - digamma(sum a) = logp_c  depends only
on logp = colmean(x) - mean_i log s_i with s_i = sum_c exp(x_ic).  The device
computes L = sum_i log s_i (data-parallel rows, 8 cores); the host does the
tiny 10-dim Newton solve plus subsampled moment estimation.

Design (v1 notes in kernel_v1_backup.py; measured numbers from NTFF traces):
- ALL input rides fp8_e4m3 (1 B/elem, 2.52 MB/core; DMA ~318 GB/s busy).
  exp splits between ScalarE exact exp (8.33 ns/col, dtype-independent) and
  VectorE Schraudolph int-exp (one tensor_scalar fp8->i16, 2x_2P mode,
  4.17 ns/col): rne_i16(x*EA+EB) IS the bf16 bit pattern of e^x.
- channel-major group tiles E[128, 10, G]: row-sum is a 4-op flat tree
  (u=E0:5+E5:10 [2.6 ns/col]; v=u0:2+u2:4 [1.04]; w=v0+v1 [.52];
  s=w+u4 [.52]), DVE tensor_tensor 2x.  Optional per-group cce flag
  replaces the 5G-wide eff-op1 with a SWDGE SBUF->SBUF DMA accumulate
  (gpsimd ring, CCE ADD in the SDMA datapath) to offload DVE.
- ON-DEVICE log: s bf16 bitcast to i16 is 128*(127+log2 s+sawtooth); one
  TensorScalarReduce (out=bf16(i*C1) dummy, accum=G*C2 + sum(i*C1), f32,
  HW-verified semantics) gives per-partition log-sums.  Whole output is
  ~3 KB/core.
- calibration WITHOUT per-row outputs: named pure-path column ranges get
  their own accumulator columns; delta_path = (device aggregate - exact
  host logsumexp over those rows)/n captures the ENTIRE pipeline bias per
  path.  Padded rows (zero tail columns of core 7; col-major row map keeps
  them whole-column) contribute a host-replicable constant per path.
- gpsimd compute is deliberately NOT used: Pool shares an SBUF port with
  DVE; a concurrent gp tensor_tensor was measured to slow DVE 2-port ops
  ~3x (probe_gp).  TensorScalarPtr is not in the Pool ISA anyway.
- fixed floor: ~4 us pre-work (iram loads/memsets/branches) + ~9.3 us
  BSP teardown barriers, invariant to kernel content (probe_empty).
"""


import numpy as np
import ml_dtypes
from contextlib import ExitStack

import concourse.bass as bass
import concourse.tile as tile
from concourse import bacc, mybir
from concourse.bass_utils import run_bass_kernel_spmd

BF16 = mybir.dt.bfloat16
F32 = mybir.dt.float32
I16 = mybir.dt.int16
FP8 = mybir.dt.float8e4
NP_BF16 = ml_dtypes.bfloat16
NP_FP8 = ml_dtypes.float8_e4m3fn

N_CORES = 8
C = 10
N_ROWS = 2_000_000

N_ITERS = 200
TOL = 1e-10
SUBSAMPLE = 10

EA = float(np.float32(128.0 / np.log(2.0)))
EB = float(np.float32(128.0 * 127.0 - 7.3365))
C1 = float(np.float32(np.log(2.0) / 128.0))
C2 = float(np.float32(-(127.0 + 0.0430) * np.log(2.0)))

ADD = mybir.AluOpType.add
MULT = mybir.AluOpType.mult


def make_geom(groups, dma_order=None, dve_order=None, act_order=None):
    """groups: dicts(G, wa, a_pieces, d_pieces, cce=False, cal=None).
    cal: list of (path, c0, c1) column ranges (group-local) that get their
    own accumulator column; path in 'A'/'D' and the range must be pure-path.
    Exactly one 'A' and one 'D' cal range must exist across all groups."""
    gs = []
    for gr in groups:
        G, wa = gr["G"], gr["wa"]
        a_pieces = gr.get("a_pieces") or ([wa] if wa else [])
        d_pieces = gr.get("d_pieces") or ([G - wa] if G - wa else [])
        assert sum(a_pieces) == wa and sum(d_pieces) == G - wa
        gs.append(dict(G=G, wa=wa, a_pieces=a_pieces, d_pieces=d_pieces,
                       cce=gr.get("cce", False), cal=gr.get("cal")))
    k = sum(g["G"] for g in gs)
    # accumulator map: per group, list of (c0, c1, acc_col, calpath|None)
    accmap = []
    col = 0
    calnames = []
    for g, gr in enumerate(gs):
        ranges = []
        cal = gr["cal"] or []
        for path, c0, c1 in cal:
            if path == "A":
                assert 0 <= c0 and c1 <= gr["wa"]
            else:
                assert gr["wa"] <= c0 and c1 <= gr["G"]
            ranges.append((c0, c1, path))
            calnames.append(path)
        covered = sorted((c0, c1) for c0, c1, _ in ranges)
        cur = 0
        rest = []
        for c0, c1 in covered:
            if c0 > cur:
                rest.append((cur, c0, None))
            cur = c1
        if cur < gr["G"]:
            rest.append((cur, gr["G"], None))
        allr = sorted(ranges + rest)
        ent = []
        for c0, c1, path in allr:
            ent.append((c0, c1, col, path))
            col += 1
        accmap.append(ent)
    assert sorted(calnames) == ["A", "D"], calnames
    ng = col
    if dma_order is None:
        dma_order = []
        for g, gr in enumerate(gs):
            for j in range(len(gr["a_pieces"])):
                dma_order.append(("a", g, j))
            for j in range(len(gr["d_pieces"])):
                dma_order.append(("d", g, j))
    if dve_order is None:
        dve_order = [("d", g, j) for g, gr in enumerate(gs)
                     for j in range(len(gr["d_pieces"]))]
        dve_order += [("t", g) for g in range(len(gs))]
    if act_order is None:
        act_order = [(g, j) for g, gr in enumerate(gs)
                     for j in range(len(gr["a_pieces"]))]
    return dict(groups=gs, k=k, rows=128 * k, ng=ng, accmap=accmap,
                dma_order=list(dma_order), dve_order=list(dve_order),
                act_order=list(act_order))


GEOM_FULL = make_geom(
    [
        dict(G=640, wa=512, a_pieces=[256, 256], d_pieces=[128],
             cal=[("A", 0, 128), ("D", 512, 640)]),
        dict(G=576, wa=448, a_pieces=[224, 224], d_pieces=[128]),
        dict(G=576, wa=448, a_pieces=[224, 224], d_pieces=[128]),
        dict(G=176, wa=176, a_pieces=[176]),
    ],
    dma_order=[
        ("a", 0, 0), ("d", 0, 0), ("a", 0, 1), ("a", 1, 0),
        ("d", 1, 0), ("a", 1, 1), ("a", 2, 0), ("d", 2, 0),
        ("a", 2, 1), ("a", 3, 0),
    ],
    dve_order=[
        ("d", 0, 0), ("d", 1, 0), ("t", 0), ("d", 2, 0),
        ("t", 1), ("t", 2), ("t", 3),
    ],
)

CFG_DEFAULT = dict()

_CACHE = {}


def emit_program(tc, ctx, aps, geom, cfg):
    nc = tc.nc
    gs = geom["groups"]
    x_d, acc_d = aps["xa"], aps["acc"]
    ng = geom["ng"]

    xa_pool = ctx.enter_context(tc.tile_pool(name="xa", bufs=1))
    e_pool = ctx.enter_context(tc.tile_pool(name="e", bufs=1))
    u_pool = ctx.enter_context(tc.tile_pool(name="u", bufs=1))
    acc_pool = ctx.enter_context(tc.tile_pool(name="acc", bufs=1))

    offs = {}
    o = 0
    for typ, g, j in geom["dma_order"]:
        w = gs[g]["a_pieces" if typ == "a" else "d_pieces"][j]
        offs[(typ, g, j)] = o
        o += 128 * C * w

    E, xt = {}, {}
    for g, gr in enumerate(gs):
        E[g] = e_pool.tile([128, C * gr["G"]], BF16, name=f"e{g}", tag=f"e{g}")
        for j, w in enumerate(gr["a_pieces"]):
            xt[("a", g, j)] = xa_pool.tile(
                [128, C * w], FP8, name=f"xa{g}_{j}", tag=f"xa{g}_{j}")
        for j, w in enumerate(gr["d_pieces"]):
            xt[("d", g, j)] = xa_pool.tile(
                [128, C * w], FP8, name=f"xd{g}_{j}", tag=f"xd{g}_{j}")
    ACC = acc_pool.tile([128, ng], F32, name="acc", tag="acc")

    # 1) input DMAs on the sync HWDGE ring in queue order
    for typ, g, j in geom["dma_order"]:
        w = gs[g]["a_pieces" if typ == "a" else "d_pieces"][j]
        o = offs[(typ, g, j)]
        src = x_d[o : o + 128 * C * w].rearrange("(p f) -> p f", f=C * w)
        nc.sync.dma_start(xt[(typ, g, j)][:], src)

    # 2) ACT exps in act_order
    for g, j in geom["act_order"]:
        gr = gs[g]
        G = gr["G"]
        w = gr["a_pieces"][j]
        o = sum(gr["a_pieces"][:j])
        E3 = E[g][:].rearrange("p (c t) -> p c t", t=G)
        nc.scalar.activation(
            E3[:, :, o : o + w],
            xt[("a", g, j)][:].rearrange("p (c t) -> p c t", t=w),
            mybir.ActivationFunctionType.Exp,
        )

    # 3) DVE program: int-exps and trees in dve_order
    for item in geom["dve_order"]:
        if item[0] == "d":
            _, g, j = item
            gr = gs[g]
            G = gr["G"]
            E3i = E[g][:].bitcast(I16).rearrange("p (c t) -> p c t", t=G)
            o = gr["wa"] + sum(gr["d_pieces"][:j])
            w = gr["d_pieces"][j]
            nc.vector.tensor_scalar(
                E3i[:, :, o : o + w],
                xt[("d", g, j)][:].rearrange("p (c t) -> p c t", t=w),
                EA, EB, op0=MULT, op1=ADD,
            )
        else:
            g = item[1]
            gr = gs[g]
            G = gr["G"]
            if gr["cce"]:
                # u = E[0:5G] += E[5G:10G] via SWDGE CCE-add (SBUF->SBUF)
                nc.gpsimd.dma_start(
                    E[g][:, 0 : 5 * G], E[g][:, 5 * G : 10 * G],
                    accum_op=ADD)
                U = E[g]
            else:
                U = u_pool.tile(
                    [128, 5 * G], BF16, name=f"u{g}", tag=f"u{g}")
                nc.vector.tensor_tensor(
                    U[:], E[g][:, 0 : 5 * G], E[g][:, 5 * G : 10 * G], op=ADD)
            # V / S / scratch share one tile [128, 4G]:
            # [0:2G]=v, [2G:3G]=s, [3G:4G]=w then intlog dummy out
            SC = u_pool.tile([128, 4 * G], BF16, name=f"sc{g}", tag=f"sc{g}")
            nc.vector.tensor_tensor(
                SC[:, 0 : 2 * G], U[:, 0 : 2 * G], U[:, 2 * G : 4 * G], op=ADD)
            nc.vector.tensor_tensor(
                SC[:, 3 * G : 4 * G], SC[:, 0:G], SC[:, G : 2 * G], op=ADD)
            nc.vector.tensor_tensor(
                SC[:, 2 * G : 3 * G], SC[:, 3 * G : 4 * G],
                U[:, 4 * G : 5 * G], op=ADD)
            for c0, c1, col, _ in geom["accmap"][g]:
                w = c1 - c0
                nc.vector.tensor_scalar(
                    SC[:, 3 * G + c0 : 3 * G + c1],
                    SC[:, 2 * G + c0 : 2 * G + c1].bitcast(I16), C1,
                    float(np.float32(w * np.float32(C2))),
                    op0=MULT, op1=ADD, accum_out=ACC[:, col : col + 1],
                )

    dst = acc_d[0 : 128 * ng].rearrange("(p f) -> p f", f=ng)
    nc.sync.dma_start(dst, ACC[:])


def build_nc(geom=None, cfg=None):
    geom = geom or GEOM_FULL
    cfg = cfg or CFG_DEFAULT
    key = str(geom) + str(cfg)
    if key in _CACHE:
        return _CACHE[key]
    nc = bacc.Bacc(
        "TRN2", target_bir_lowering=False, debug=False, num_devices=N_CORES
    )
    ntot = 128 * C * geom["k"]
    aps = {
        "xa": nc.dram_tensor("xa", [ntot], FP8, kind="ExternalInput").ap(),
        "acc": nc.dram_tensor(
            "acc", [128 * geom["ng"]], F32, kind="ExternalOutput").ap(),
    }
    with tile.TileContext(nc) as tc, ExitStack() as ctx:
        emit_program(tc, ctx, aps, geom, cfg)
    nc.compile()
    _CACHE[key] = nc
    return nc


def shard_starts(n_rows, geom):
    r = geom["rows"]
    return [min(i * r, n_rows) for i in range(N_CORES)]


def pack_core(x, start, geom):
    gs = geom["groups"]
    k, r = geom["k"], geom["rows"]
    n_real = min(r, max(0, x.shape[0] - start))
    xr = np.zeros((r, C), dtype=np.float32)
    xr[:n_real] = x[start : start + n_real]
    # col-major row map: row = c*128 + p -> x3[p, ch, c]
    x3 = np.ascontiguousarray(xr.reshape(k, 128, C).transpose(1, 2, 0))
    goff = np.cumsum([0] + [g["G"] for g in gs])
    chunks = []
    for typ, g, j in geom["dma_order"]:
        gr = gs[g]
        if typ == "a":
            o = goff[g] + sum(gr["a_pieces"][:j])
            w = gr["a_pieces"][j]
        else:
            o = goff[g] + gr["wa"] + sum(gr["d_pieces"][:j])
            w = gr["d_pieces"][j]
        chunks.append(np.ascontiguousarray(x3[:, :, o : o + w]).reshape(-1))
    xa = np.concatenate(chunks).astype(NP_FP8)
    return xa, n_real


def digamma(x):
    x = np.asarray(x, dtype=np.float64)
    res = np.zeros_like(x)
    for i in range(8):
        res -= 1.0 / (x + i)
    y = x + 8.0
    y2 = 1.0 / (y * y)
    res += (
        np.log(y)
        - 0.5 / y
        - y2
        * (
            1.0 / 12
            - y2 * (1.0 / 120 - y2 * (1.0 / 252 - y2 * (1.0 / 240 - y2 / 132)))
        )
    )
    return res


def trigamma(x):
    x = np.asarray(x, dtype=np.float64)
    res = np.zeros_like(x)
    for i in range(8):
        res += 1.0 / (x + i) ** 2
    y = x + 8.0
    y2 = 1.0 / (y * y)
    res += (
        1.0 / y
        + 0.5 * y2
        + y2
        / y
        * (1.0 / 6 - y2 * (1.0 / 30 - y2 * (1.0 / 42 - y2 * (1.0 / 30 - y2 * 5.0 / 66))))
    )
    return res


def newton(m1, m2, logp, n):
    a = m1 * (((m1 - m2) / (m2 - m1 * m1)).mean())
    a = np.maximum(a, 1e-6)
    for _ in range(N_ITERS):
        asum = a.sum()
        g = (digamma(asum) - digamma(a) + logp) * n
        q = -n * trigamma(a)
        z = n * trigamma(asum)
        qi = 1.0 / q
        b = (g * qi).sum() / (1.0 / z + qi.sum())
        a_new = a - (g - b) * qi
        a_new = np.maximum(a_new, 1e-8)
        diff = np.abs(a_new - a).sum()
        a = a_new
        if diff < TOL:
            break
    return a


def run_device(x, geom=None, cfg=None, trace=False, **kw):
    geom = geom or GEOM_FULL
    cfg = cfg or CFG_DEFAULT
    nc = build_nc(geom, cfg)
    starts = shard_starts(x.shape[0], geom)
    in_maps = []
    for i in range(N_CORES):
        xa, _ = pack_core(x, starts[i], geom)
        in_maps.append({"xa": xa})
    res = run_bass_kernel_spmd(
        nc, in_maps, core_ids=list(range(N_CORES)), trace=trace, **kw
    )
    return res


def _bf16(x):
    return np.asarray(x, np.float32).astype(NP_BF16).astype(np.float32)


def pad_constants():
    """intlog of the padded-row (x=0) s per path, replicating device math."""
    eA = np.float32(1.0)
    iD = np.int16(np.rint(np.float32(0.0) * np.float32(EA) + np.float32(EB)))
    eD = iD.view(NP_BF16).astype(np.float32)
    out = {}
    for name, e in [("A", eA), ("D", eD)]:
        u = _bf16(e + e)
        v = _bf16(u + u)
        w = _bf16(v + v)
        s = _bf16(w + u)
        i = np.float32(np.asarray(s, np.float32).astype(NP_BF16).view(np.int16))
        out[name] = float(np.float32(i * np.float32(C1)) + np.float32(C2))
    return out


def finish_host(x, results, geom=None):
    geom = geom or GEOM_FULL
    gs = geom["groups"]
    ng = geom["ng"]
    n = x.shape[0]
    starts = shard_starts(n, geom)
    pc = pad_constants()
    goff = np.cumsum([0] + [g["G"] for g in gs])

    # per-acc-col global metadata: (global col range, per-col path array)
    colmeta = []
    for g, gr in enumerate(gs):
        pathv = np.array([True] * gr["wa"] + [False] * (gr["G"] - gr["wa"]))
        for c0, c1, col, calpath in geom["accmap"][g]:
            colmeta.append((goff[g] + c0, goff[g] + c1, pathv[c0:c1], calpath))
    colmeta.sort(key=lambda t: t[0])

    L = 0.0
    n_A = 0
    n_D = 0
    cal_sums = {"A": 0.0, "D": 0.0}
    cal_rows = {"A": 0, "D": 0}
    for i in range(N_CORES):
        acc = np.asarray(results[i]["acc"]).reshape(128, ng)
        n_real = min(geom["rows"], max(0, n - starts[i]))
        rc = n_real // 128
        assert rc * 128 == n_real, (i, n_real)
        for g, gr in enumerate(gs):
            for c0, c1, col, calpath in geom["accmap"][g]:
                gc0, gc1 = goff[g] + c0, goff[g] + c1
                pathv = np.array(
                    [True] * gr["wa"] + [False] * (gr["G"] - gr["wa"]))[c0:c1]
                csum = float(acc[:, col].sum(dtype=np.float64))
                cols_real = np.arange(gc0, gc1) < rc
                pa = int((~cols_real & pathv).sum())
                pd = int((~cols_real & ~pathv).sum())
                csum -= 128 * (pa * pc["A"] + pd * pc["D"])
                L += csum
                ra = int((cols_real & pathv).sum())
                rd = int((cols_real & ~pathv).sum())
                n_A += 128 * ra
                n_D += 128 * rd
                if calpath:
                    cal_sums[calpath] += csum
                    cal_rows[calpath] += 128 * (ra + rd)
    assert n_A + n_D == n, (n_A, n_D)

    deltas = {}
    for g, gr in enumerate(gs):
        for c0, c1, col, calpath in geom["accmap"][g]:
            if not calpath:
                continue
            gc0 = goff[g] + c0
            rows = []
            for i in range(N_CORES):
                st = starts[i] + gc0 * 128
                rows.append(x[st : st + 128 * (c1 - c0)])
            xr = np.concatenate(rows).astype(np.float64)
            m = xr.max(axis=1, keepdims=True)
            ls = np.log(np.exp(xr - m).sum(axis=1)) + m[:, 0]
            deltas[calpath] = cal_sums[calpath] / cal_rows[calpath] - ls.mean()
    L_corr = L - n_A * deltas["A"] - n_D * deltas["D"]

    xsum = x.sum(axis=0, dtype=np.float64)
    logp = xsum / n - L_corr / n

    xm = x[::SUBSAMPLE].astype(np.float64)
    es = np.exp(xm - xm.max(axis=1, keepdims=True))
    ps = es / es.sum(axis=1, keepdims=True)
    m1 = ps.mean(0)
    m2 = (ps * ps).mean(0)
    a = newton(m1, m2, logp, float(n))
    return a.astype(np.float32)


def kernel(x):
    x = np.asarray(x)
    assert x.shape == (N_ROWS, C) and x.dtype == np.float32, (x.shape, x.dtype)
    res = run_device(x)
    return finish_host(x, res.results)
